# revision 12
# baseline (speedup 1.0000x reference)
"""GCN conv (linear -> weighted gather -> segment-sum by dst) on 8 trn2 cores.

Math: out = segment_sum((x @ W.T + b)[src] * w[:, None], dst, N)
    = segment_sum(w*[x|1], dst) @ [W|b].T   (linear applied post-aggregation)

Strategy (v2; v1 used 1274 per-chunk indirect DMAs at ~1us Q7 fixed cost each):
  - Nodes range-partitioned over cores; each core's edges grouped into 98
    blocks of 128 dst nodes. Each block's edges are split into 4 "buckets"
    so src indices fit int16: bucket k's rows live in a 32768-row window of
    a per-core fp16 table x2[4*32768, 128] = [x | 1 | pad] (unique srcs per
    (core, bucket) are ~20k < 32768).
  - The per-block chunk schedule (how many 128-edge chunks each (block,
    bucket) cell gets) is computed from the run's data: per block, the max
    load across cores rounded up to chunks, split across buckets. The same
    schedule is used by every core (SPMD), so per-core padding is only the
    cross-core load spread (~3%) plus chunk rounding.
  - Device gathers each group's (14 blocks) bucket slots with ONE
    InstDMAGatherAnt per (group, bucket): ~28 gather instructions total
    (vs 1274), each emitting per-row 132B descriptors (fp16 rows).
  - Per chunk: a one-hot weighted matmul (fp16) accumulates the pre-linear
    segment sum in PSUM; per block: a second matmul applies [W|b].
"""

import bass_rust
import numpy as np

from concourse import ap_utils, bass, library_config, mybir, tile
from concourse.bass_utils import run_bass_kernel_spmd
from concourse.library_overlay import lower_extended_insts
from concourse._compat import exact_div

P = 128
NCORES = 8
N, E, D = 100000, 1200000, 64
NODES_PER_CORE = N // NCORES  # 12500
NB = (NODES_PER_CORE + P - 1) // P  # 98 blocks of 128 dst nodes
NPAD = NB * P  # 12544
NBUCK = 4
WIN = 32768  # x2 rows per bucket window (int16-indexable)
EL = D + 2  # gathered row: 64 feats + ones col + pad (4B-aligned descs)
ST = 2 * D  # x2 row stride in fp16 elements (256B, required by dma_gather)
# gather group sizes: small first group (compute starts early), large middle
# groups (amortize per-call overhead), tiny last groups (short compute tail)
GROUP_SIZES = [4, 7] + [8] * 10 + [4, 3]
assert sum(GROUP_SIZES) == NB
GROUP_STARTS = [sum(GROUP_SIZES[:i]) for i in range(len(GROUP_SIZES))]
NG = len(GROUP_SIZES)

f16 = mybir.dt.float16
f32 = mybir.dt.float32
i16 = mybir.dt.int16

_wait_counter = [0]


def _split_multi_waits(nc):
    """Installed walrus rejects >1 sync wait per instruction; park excess
    waits on fresh single-wait NoOps inserted before the owner (same
    engine, so in-order execution preserves semantics)."""
    for fn in nc.m.functions:
        for bb in fn.blocks:
            insts = bb.instructions
            if not any(
                i.sync_info is not None and len(i.sync_info.on_wait) > 1
                for i in insts
            ):
                continue
            out = []
            for inst in insts:
                si = inst.sync_info
                waits = list(si.on_wait) if si is not None else []
                if len(waits) > 1:
                    for wv in waits[:-1]:
                        _wait_counter[0] += 1
                        nop = mybir.InstNoOp(
                            name=f"waitsplit-{_wait_counter[0]}",
                            engine=inst.engine,
                        )
                        nop.sync_info = bass_rust.SyncInfo(
                            on_wait=[wv], on_update=[]
                        )
                        out.append(nop)
                    inst.sync_info = bass_rust.SyncInfo(
                        on_wait=[waits[-1]], on_update=list(si.on_update)
                    )
                out.append(inst)
            bb.instructions = out


class _TC(tile.TileContext):
    def __exit__(self, *args):
        ret = super().__exit__(*args)
        _split_multi_waits(self.nc)
        return ret


_REG_CACHE = {}


def _num_idxs_reg(eng, num_idxs):
    key = (id(eng.bass), num_idxs)
    if key not in _REG_CACHE:
        _REG_CACHE[key] = eng.to_reg(num_idxs)
    return _REG_CACHE[key]


def _dma_gather_raw(eng, out_ap, in_ap, idxs_ap, num_idxs, elem_size, elem_step,
                    single_packet=True, queue_num=0):
    """bass.BassGpSimd.dma_gather (HBM source, non-transpose) without the
    elem_size_bytes%256 restriction: only the row *stride* must be a
    multiple of 256B; the Q7 kernel emits arbitrary-length descriptors."""
    eng._assert_queue_num(queue_num)
    assert idxs_ap.dtype == mybir.dt.int16
    assert in_ap.dtype == out_ap.dtype
    elem_size_bytes = elem_size * mybir.dt.size(in_ap.dtype)
    assert elem_size_bytes > 0 and elem_size_bytes % 4 == 0
    assert in_ap.space == bass.MemorySpace.DRAM
    assert idxs_ap.space == bass.MemorySpace.SBUF
    assert out_ap.space == bass.MemorySpace.SBUF
    assert ap_utils.ap_is_contiguous(out_ap.ap[1:])
    assert ap_utils.ap_is_contiguous(idxs_ap.ap[1:])
    assert in_ap.ap[-1][1] == out_ap.ap[-1][1] == elem_size
    assert out_ap.ap[0][1] * out_ap.ap[1][1] == num_idxs
    assert num_idxs % P == 0
    assert in_ap.ap[0][0] == elem_step
    stride_bytes_256 = exact_div(elem_step * mybir.dt.size(in_ap.dtype), 256)
    assert stride_bytes_256 < 256
    _in_ap = eng.lower_ap_dma(in_ap, for_custom_bir_dma=True)
    _idxs_ap = eng.lower_ap(idxs_ap)
    _out_ap = eng.lower_ap(out_ap)
    return eng.add_instruction(
        mybir.InstDMAGatherAnt(
            name=eng.bass.get_next_instruction_name(),
            ins=[*_in_ap, _idxs_ap, eng.lower_val_access(_num_idxs_reg(eng, num_idxs))],
            outs=[_out_ap],
            transpose=False,
            num_idxs=num_idxs,
            elem_size=elem_size,
            stride_bytes_256=stride_bytes_256,
            gen_mode=0,
            single_packet=single_packet,
            queue_num=queue_num,
            sbuf_tokens_per_rank=0,
            sbuf_free_dim_per_rank=0,
            sbuf_free_dim_pad_per_rank=0,
            sbuf_byte_offset=0,
        )
    )


class _Schedule:
    """Shared (SPMD) per-block chunk layout computed from the run's data."""

    def __init__(self, maxload):
        # chunks per block: fit the largest core's load, >= 1
        self.Bb = np.maximum(1, -(-maxload // P)).astype(np.int64)  # [NB]
        self.Tb = np.zeros(NB + 1, dtype=np.int64)
        self.Tb[1:] = np.cumsum(self.Bb)
        self.nchunks = np.zeros((NB, NBUCK), dtype=np.int64)
        for b in range(NB):
            base, rem = divmod(int(self.Bb[b]), NBUCK)
            self.nchunks[b, :] = base
            for i in range(rem):
                self.nchunks[b, (b + i) % NBUCK] += 1
        self.NCHUNKS = int(self.Tb[-1])
        self.SLOTS = self.NCHUNKS * P
        self.IDX_COLS = self.SLOTS // 16
        # chunk offset of cell (b, k): a block's chunks laid out k-ascending
        self.cell_chunk0 = np.zeros((NB, NBUCK), dtype=np.int64)
        for b in range(NB):
            ofs = 0
            for k in range(NBUCK):
                self.cell_chunk0[b, k] = self.Tb[b] + ofs
                ofs += self.nchunks[b, k]


def _build_program(sch: _Schedule):
    nc = bass.Bass(num_swdge_queues=4)
    x2_p = nc.declare_dram_parameter("x2", [NBUCK * WIN, ST], f16, isOutput=False)
    idx_p = nc.declare_dram_parameter("idxw", [P, sch.IDX_COLS], i16, isOutput=False)
    ohT_p = nc.declare_dram_parameter("ohT", [P, sch.NCHUNKS, P], f16, isOutput=False)
    wext_p = nc.declare_dram_parameter("wext", [D + 1, D], f16, isOutput=False)
    out_p = nc.declare_dram_parameter("out", [NPAD, D], f32, isOutput=True)

    nc.gpsimd.load_library(library_config.mlp)
    with _TC(nc) as tc:
        with (
            tc.tile_pool(name="const", bufs=1) as cpool,
            tc.tile_pool(name="gx", bufs=6 * NBUCK) as gxpool,
            tc.tile_pool(name="oh", bufs=8) as ohpool,
            tc.tile_pool(name="stsb", bufs=2) as stpool,
            tc.tile_pool(name="outsb", bufs=3) as opool,
            tc.tile_pool(name="pst", bufs=3, space="PSUM") as pstpool,
            tc.tile_pool(name="pout", bufs=3, space="PSUM") as poutpool,
        ):
            wext_sb = cpool.tile([D + 1, D], f16)
            nc.sync.dma_start(out=wext_sb[:], in_=wext_p[:])
            idx_sb = cpool.tile([P, sch.IDX_COLS], i16)
            # load the per-(group,bucket) idx segments separately so the
            # first gathers don't wait on the whole table
            seg_cols = []
            off = 0
            for g in range(NG):
                g0, g1 = GROUP_STARTS[g], GROUP_STARTS[g] + GROUP_SIZES[g]
                for k in range(NBUCK):
                    nch = int(sch.nchunks[g0:g1, k].sum())
                    cols = nch * P // 16
                    seg_cols.append((off, cols))
                    if cols:
                        nc.sync.dma_start(
                            out=idx_sb[:, off : off + cols],
                            in_=idx_p[:, off : off + cols],
                        )
                    off += cols
            assert off == sch.IDX_COLS

            seg_i = 0
            for g in range(NG):
                g0, g1 = GROUP_STARTS[g], GROUP_STARTS[g] + GROUP_SIZES[g]
                blocks = range(g0, g1)
                gx = {}
                cmap = {}
                for k in range(NBUCK):
                    nch = int(sch.nchunks[g0:g1, k].sum())
                    off, cols = seg_cols[seg_i]
                    seg_i += 1
                    if nch == 0:
                        continue
                    t = gxpool.tile([P, nch, EL], f16)
                    _dma_gather_raw(
                        nc.gpsimd,
                        out_ap=t[:, :, :],
                        in_ap=x2_p[k * WIN : (k + 1) * WIN, 0:EL],
                        idxs_ap=idx_sb[:, off : off + cols],
                        num_idxs=nch * P,
                        elem_size=EL,
                        elem_step=ST,
                        single_packet=False,
                        queue_num=k,
                    )
                    gx[k] = t
                    c = 0
                    for b in blocks:
                        cmap[(b, k)] = c
                        c += int(sch.nchunks[b, k])
                for b in blocks:
                    pst = pstpool.tile([D + 1, P], f32)
                    seq = [
                        (k, j)
                        for k in range(NBUCK)
                        for j in range(int(sch.nchunks[b, k]))
                    ]
                    # oh[p, i, f] = w[p] * (rel_dst[p] == f), host-precomputed
                    oht = ohpool.tile([P, len(seq), P], f16)
                    nc.sync.dma_start(
                        out=oht[:, :, :],
                        in_=ohT_p[:, int(sch.Tb[b]) : int(sch.Tb[b + 1]), :],
                    )
                    for i, (k, j) in enumerate(seq):
                        # pst[feat, node] += sum_p gx[p, feat] * oh[p, i, node]
                        nc.tensor.matmul(
                            pst[:],
                            lhsT=gx[k][:, cmap[(b, k)] + j, 0 : D + 1],
                            rhs=oht[:, i, :],
                            start=(i == 0),
                            stop=(i == len(seq) - 1),
                        )
                    st_sb = stpool.tile([D + 1, P], f16)
                    nc.any.tensor_copy(out=st_sb[:], in_=pst[:])
                    pout = poutpool.tile([P, D], f32)
                    # out[node, dout] = sum_k st[k, node] * wext[k, dout]
                    nc.tensor.matmul(
                        pout[:], lhsT=st_sb[:], rhs=wext_sb[:], start=True, stop=True
                    )
                    out_sb = opool.tile([P, D], f32)
                    nc.any.tensor_copy(out=out_sb[:], in_=pout[:])
                    nc.sync.dma_start(
                        out=out_p[b * P : (b + 1) * P, :], in_=out_sb[:]
                    )
    lower_extended_insts(nc)
    return nc


def _wrap_idx_segments(sch: _Schedule, slot_uid):
    """Reorder block-major slot uids into the device idx table
    [P, IDX_COLS]: per (group, bucket) call, concatenated cell slots
    wrapped 16-wide and replicated across the 8 Q7 partition groups."""
    out = np.zeros((P, sch.IDX_COLS), dtype=np.int16)
    col = 0
    for g in range(NG):
        g0, g1 = GROUP_STARTS[g], GROUP_STARTS[g] + GROUP_SIZES[g]
        for k in range(NBUCK):
            segs = []
            for b in range(g0, g1):
                s0 = sch.cell_chunk0[b, k] * P
                segs.append(slot_uid[s0 : s0 + sch.nchunks[b, k] * P])
            seg = np.concatenate(segs)
            n = len(seg)
            if n == 0:
                continue
            wv = np.zeros((16, n // 16), dtype=np.int16)
            wv[np.arange(n) % 16, np.arange(n) // 16] = seg
            for rep in range(8):
                out[16 * rep : 16 * (rep + 1), col : col + n // 16] = wv
            col += n // 16
    assert col == sch.IDX_COLS
    return out


def kernel(x, src, dst, w, W, b):
    x = np.asarray(x, dtype=np.float32)
    src = np.asarray(src).astype(np.int64)
    dst = np.asarray(dst).astype(np.int64)
    w = np.asarray(w, dtype=np.float32)
    W = np.asarray(W, dtype=np.float32)
    b = np.asarray(b, dtype=np.float32)

    x16 = x.astype(np.float16)
    wext = np.zeros((D + 1, D), dtype=np.float16)
    wext[:D] = W.T.astype(np.float16)
    wext[D] = b.astype(np.float16)

    core_of = dst // NODES_PER_CORE
    percore = []
    loads = np.zeros((NCORES, NB), dtype=np.int64)
    for c in range(NCORES):
        m = core_of == c
        s_c = src[m]
        d_c = dst[m] - c * NODES_PER_CORE
        w_c = w[m].astype(np.float16)
        blk = d_c >> 7
        order = np.lexsort((s_c, blk))
        s_c, d_c, w_c, blk = s_c[order], d_c[order], w_c[order], blk[order]
        counts = np.bincount(blk, minlength=NB)
        loads[c] = counts
        percore.append((s_c, d_c, w_c, counts))

    sch = _Schedule(loads.max(axis=0))

    in_maps = []
    for c in range(NCORES):
        s_c, d_c, w_c, counts = percore[c]
        starts = np.zeros(NB + 1, dtype=np.int64)
        starts[1:] = np.cumsum(counts)

        # per block: split the src-sorted run into bucket cells (balanced,
        # capped by the shared schedule); record per-edge slot positions
        slot_of_edge = np.empty(len(s_c), dtype=np.int64)
        bucket_of_edge = np.empty(len(s_c), dtype=np.int8)
        for bb in range(NB):
            L = int(counts[bb])
            caps = sch.nchunks[bb] * P
            fair = L // NBUCK
            n = np.minimum(caps, fair)
            rem = L - int(n.sum())
            for k in range(NBUCK):
                if rem <= 0:
                    break
                add = min(int(caps[k] - n[k]), rem)
                n[k] += add
                rem -= add
            assert rem == 0, (c, bb, L, caps)
            e0 = starts[bb]
            for k in range(NBUCK):
                cnt = int(n[k])
                cell_slot0 = sch.cell_chunk0[bb, k] * P
                slot_of_edge[e0 : e0 + cnt] = cell_slot0 + np.arange(cnt)
                bucket_of_edge[e0 : e0 + cnt] = k
                e0 += cnt

        # per bucket: unique srcs -> window-local uids; fill x2 + slot arrays
        x2 = np.zeros((NBUCK * WIN, ST), dtype=np.float16)
        slot_uid = np.zeros(sch.SLOTS, dtype=np.int16)
        for k in range(NBUCK):
            em = bucket_of_edge == k
            uniq, inv = np.unique(s_c[em], return_inverse=True)
            assert len(uniq) <= WIN, (c, k, len(uniq))
            x2[k * WIN : k * WIN + len(uniq), 0:D] = x16[uniq]
            x2[k * WIN : k * WIN + len(uniq), D] = np.float16(1.0)
            slot_uid[slot_of_edge[em]] = inv.astype(np.int16)
        # one-hot tables: oh[slot, f] = w * (rel_dst == f), zero for pad slots
        oh_flat = np.zeros((sch.SLOTS, P), dtype=np.float16)
        oh_flat[slot_of_edge, (d_c & 127).astype(np.int64)] = w_c
        ohT = np.ascontiguousarray(
            np.transpose(oh_flat.reshape(sch.NCHUNKS, P, P), (1, 0, 2))
        )

        in_maps.append(
            {
                "x2": x2,
                "idxw": _wrap_idx_segments(sch, slot_uid),
                "ohT": ohT,
                "wext": wext,
            }
        )

    nc = _build_program(sch)
    global _last_nc, _last_in_maps
    _last_nc, _last_in_maps = nc, in_maps
    results = run_bass_kernel_spmd(nc, in_maps, list(range(NCORES))).results
    out = np.concatenate(
        [results[c]["out"][:NODES_PER_CORE] for c in range(NCORES)], axis=0
    )
    return out.astype(np.float32)
